# revision 15
# baseline (speedup 1.0000x reference)
"""Deformable attention Bass kernel for Trainium2, sharded over 8 NeuronCores.

Problem: nn_DeformableAttention (Q=40000 queries, C=256, 2 levels of 128x256
feature maps, 1 head, 4 points/level, bilinear grid-sample w/ zero padding).

Strategy (v2): queries sharded across 8 cores (5000 each, padded to 5120 = 40
tiles of 128); value maps + linears replicated. The key bottleneck of v1 was
SWDGE descriptor generation on the Pool engine: 32 indirect DMAs per tile at
~1.1us fixed cost each. v2 cuts this to 8 per tile by gathering 2KB "quad"
rows from a host-built bf16 buffer valq[l*NV + y*W + x] =
[v(y,x), v(y,x+1), v(y+1,x), v(y+1,x+1)]  (all 4 bilinear corners of one
point in one row). The gather lands corner-partitioned (partition = 16*j2+g,
j2 = (level,point), g = query%16), so the weighted combine runs on the PE as
32 small matmuls accumulating in f32 PSUM; the per-query bilinear weights are
spread into the block-diagonal lhsT layout with two PE transposes plus
constant spread/mask matrices.

Per 128-query tile:
  - PE: offset/attention linears (bf16), two transposes (x0y coords, U
    weights), offset-row matmuls, 4 weight-spread matmuls, 32 combine matmuls
  - DVE/ACT: coords, clamped corner indices, hat weights, softmax, mask-mults
  - Pool: 8 indirect DMA gathers (one per (level,point)), 2KB rows
"""
import sys
import os

sys.path.insert(0, '/opt/trn_rl_repo')

import numpy as np

import concourse.bass as bass
import concourse.mybir as mybir
from concourse.bass import ts
from concourse.tile import TileContext

F32 = mybir.dt.float32
BF16 = mybir.dt.bfloat16
I32 = mybir.dt.int32

N_CORES = 8
H, W = 128, 256
C = 256
NV = H * W          # rows per level feature map
QTOT = 40000
QPC = QTOT // N_CORES       # 5000 queries per core
P = 128                     # partition/tile size
NT_FULL = (QPC + P - 1) // P  # 40 tiles (last padded)
QPAD = NT_FULL * P          # 5120

# consf (f32) column layout: smat(4) | b_all(24) | cmax(16) | maskg(128)
NCF = 4 + 24 + 16 + 128
# consb (bf16) column layout: identity(128) | S8(8*128) | M0(128) | M1(128) |
#                             nvall(128) | onesb(128)
NCB = 128 + 8 * 128 + 128 + 128 + 128 + 128

_WAIT_OP_FROM_MODE = {
    "sem-ge-imm": "sem-ge",
    "sem-eq-imm": "sem-eq",
    "sem-ge": "sem-ge",
    "sem-eq": "sem-eq",
}


def _split_multiwait_noctrl(nc, max_waits=1):
    """This walrus build rejects >1 sync-wait per instruction ("Too many sync
    wait commands"). Hoist extra waits onto standalone single-wait
    EventSemaphore instructions placed immediately before, on the same engine
    (program order on the engine queue preserves semantics)."""
    import bass_rust

    for f in nc.m.functions:
        for b in f.blocks:
            il = list(b.instructions)
            need = [i for i in il
                    if i.sync_info is not None
                    and len(i.sync_info.on_wait) > max_waits]
            if not need:
                continue
            carriers = {}
            created = []
            for inst in need:
                waits = list(inst.sync_info.on_wait)
                cs = []
                for wt in waits[max_waits:]:
                    h = bass_rust.SemaphoreHandle(wt.ant_name, wt.id)
                    ev = nc.engines[inst.engine].wait_op(
                        h, wt.wait_value, _WAIT_OP_FROM_MODE[wt.wait_mode])
                    cs.append(ev.ins)
                    created.append(ev.ins)
                si = inst.sync_info
                si.on_wait = waits[:max_waits]
                inst.sync_info = si
                carriers[inst.name] = cs
            # the new instructions were appended to nc.cur_bb; remove them
            # from wherever they landed, then splice before their drains.
            created_names = {i.name for i in created}
            for f2 in nc.m.functions:
                for b2 in f2.blocks:
                    lst = list(b2.instructions)
                    kept = [i for i in lst if i.name not in created_names]
                    if len(kept) != len(lst):
                        b2.instructions = kept
            out = []
            for inst in list(b.instructions):
                out.extend(carriers.get(inst.name, []))
                out.append(inst)
            b.instructions = out


# cast_mode: 'rne' = hw f32->int32 cast rounds to nearest; 'trunc' = truncates
CAST_MODE = os.environ.get('DEFATT_CAST_MODE', 'rne')


def build_nc(n_tiles=NT_FULL, gather_bufs=6, work_bufs=5, debug=False):
    qpad = n_tiles * P
    nc = bass.Bass("TRN2")
    dbg = {}
    if debug:
        for nm, shp, dt_ in [("d_lin", [P, 24], F32), ("d_iall", [P, 16], F32),
                             ("d_x0f", [P, 16], F32), ("d_U", [P, 32], F32),
                             ("d_offf", [P, 8], F32), ("d_xT", [16, P], F32),
                             ("d_UT", [32, P], F32), ("d_Lb", [P, 8 * P], F32),
                             ("d_G", [P, 8 * 1024], F32)]:
            dbg[nm] = nc.dram_tensor(nm, shp, dt_, kind="ExternalOutput")

    qryT = nc.dram_tensor("qryT", [C, qpad], BF16, kind="ExternalInput")
    refp = nc.dram_tensor("refp", [qpad, 4], F32, kind="ExternalInput")
    valq = nc.dram_tensor("valq", [2 * NV, 4 * C], BF16, kind="ExternalInput")
    wall = nc.dram_tensor("wall", [C, 24], BF16, kind="ExternalInput")
    consf = nc.dram_tensor("consf", [P, NCF], F32, kind="ExternalInput")
    consb = nc.dram_tensor("consb", [P, NCB], BF16, kind="ExternalInput")
    out = nc.dram_tensor("out", [qpad, C], F32, kind="ExternalOutput")

    with TileContext(nc) as tc:
        with (
            tc.tile_pool(name="const", bufs=1) as cp,
            tc.tile_pool(name="work", bufs=work_bufs) as wp,
            tc.tile_pool(name="gather", bufs=gather_bufs) as gp,
            tc.tile_pool(name="ps1", bufs=1, space="PSUM") as pp1,
            tc.tile_pool(name="ps2", bufs=2, space="PSUM") as pp2,
        ):
            # ---- constants, loaded once ----
            ctf = cp.tile([P, NCF], F32)
            nc.sync.dma_start(ctf[:], consf[:, :])
            smat = ctf[:, 0:4]            # [W, H, W, H]
            b_all = ctf[:, 4:28]          # bias (b_off || b_attn) bcast
            cmax = ctf[:, 28:44]          # clamp max per (l,p,xy): x->W-2, y->H-2
            maskg = ctf[:, 44:172]        # [128,128] delta(p%32 == col%32)

            ctb = cp.tile([P, NCB], BF16)
            nc.sync.dma_start(ctb[:], consb[:, :])
            idb = ctb[:, 0:128]                      # identity for PE transpose
            S8 = ctb[:, 128:128 + 8 * 128]           # 8 spread matrices [32,128]
            M0 = ctb[0:16, 1152:1280]                # [16,128] offset rows l=0
            M1 = ctb[0:16, 1280:1408]                # [16,128] offset rows l=1
            nvall = ctb[0:1, 1408:1536]              # [1,128] all NV
            onesb = ctb[0:1, 1536:1664]              # [1,128] ones

            wt = cp.tile([P, 2, 24], BF16)   # W_all split into two K-chunks
            nc.sync.dma_start(
                wt[:], wall.rearrange("(h p) n -> p h n", p=P)[:, :, :])

            qryT_r = qryT.rearrange("(h p) q -> p h q", p=P)

            for t in range(n_tiles):
                # ---- load query (pre-transposed, bf16) + reference points ----
                qT = wp.tile([P, 2, P], BF16, tag="qT")
                nc.sync.dma_start(qT[:], qryT_r[:, :, ts(t, P)])
                rt = wp.tile([P, 4], F32, tag="rt")
                nc.sync.dma_start(rt[:], refp[ts(t, P), :])

                # ---- linears: lin = q @ [W_off || W_attn] + b ----
                lin_ps = pp1.tile([P, 24], F32, tag="lin")
                nc.tensor.matmul(out=lin_ps[:], lhsT=qT[:, 0, :],
                                 rhs=wt[:, 0, :], start=True, stop=False)
                nc.tensor.matmul(out=lin_ps[:], lhsT=qT[:, 1, :],
                                 rhs=wt[:, 1, :], start=False, stop=True)
                lin = wp.tile([P, 24], F32, tag="lin_sb")
                nc.vector.tensor_add(out=lin[:], in0=lin_ps[:], in1=b_all)

                # ---- sample coords: i = ref*scale + off - 0.5 ----
                refsc = wp.tile([P, 4], F32, tag="refsc")
                nc.vector.tensor_mul(out=refsc[:], in0=rt[:], in1=smat)
                i_all = wp.tile([P, 16], F32, tag="i_all")
                for l in range(2):
                    refsc_b = refsc[:, 2 * l:2 * l + 2] \
                        .unsqueeze(1).broadcast_to([P, 4, 2])
                    nc.vector.scalar_tensor_tensor(
                        out=i_all[:, 8 * l:8 * l + 8]
                            .rearrange("p (k x) -> p k x", k=4),
                        in0=lin[:, 8 * l:8 * l + 8]
                            .rearrange("p (k x) -> p k x", k=4),
                        scalar=-0.5, in1=refsc_b,
                        op0=mybir.AluOpType.add, op1=mybir.AluOpType.add)

                # ---- low corner: x0 = clamp(round(i - 0.5), 0, {W,H}-2) ----
                t2 = wp.tile([P, 16], F32, tag="t2")
                nc.vector.tensor_scalar(
                    out=t2[:], in0=i_all[:], scalar1=-0.5, scalar2=0.0,
                    op0=mybir.AluOpType.add, op1=mybir.AluOpType.max)
                nc.vector.tensor_tensor(out=t2[:], in0=t2[:], in1=cmax,
                                        op=mybir.AluOpType.min)
                if CAST_MODE == 'trunc':
                    nc.vector.tensor_scalar_add(out=t2[:], in0=t2[:],
                                                scalar1=0.5)
                x0i = wp.tile([P, 16], I32, tag="x0i")
                nc.vector.tensor_copy(out=x0i[:], in_=t2[:])
                x0f = wp.tile([P, 16], F32, tag="x0f")
                nc.vector.tensor_copy(out=x0f[:], in_=x0i[:])

                # ---- transpose x0y into feature-partitioned layout ----
                x0b = wp.tile([P, 16], BF16, tag="x0b")
                nc.scalar.activation(x0b[:], x0f[:],
                                     mybir.ActivationFunctionType.Copy)
                trx = pp1.tile([16, P], BF16, tag="trx")
                nc.tensor.transpose(out=trx[:], in_=x0b[:], identity=idb)
                xT = wp.tile([16, P], BF16, tag="xT")
                nc.vector.tensor_copy(out=xT[:], in_=trx[:])

                # ---- offsets: offfull_l[32pt+g, q] = l*NV + y0*W + x0 ----
                off_ps = pp1.tile([P, 2, P], F32, tag="off")
                nc.tensor.matmul(out=off_ps[:, 0, :], lhsT=M0, rhs=xT[:],
                                 start=True, stop=True)
                nc.tensor.matmul(out=off_ps[:, 1, :], lhsT=M1, rhs=xT[:],
                                 start=True, stop=False)
                nc.tensor.matmul(out=off_ps[:, 1, :], lhsT=nvall, rhs=onesb,
                                 start=False, stop=True)
                off_f = wp.tile([P, 8], F32, tag="off_f")
                for l in range(2):
                    offm = wp.tile([P, P], F32, tag="offm")
                    nc.vector.tensor_mul(out=offm[:], in0=off_ps[:, l, :],
                                         in1=maskg)
                    nc.vector.tensor_reduce(
                        out=off_f[:, 4 * l:4 * (l + 1)],
                        in_=offm[:].rearrange("p (c g) -> p c g", c=4),
                        axis=mybir.AxisListType.X, op=mybir.AluOpType.add)
                off32 = wp.tile([P, 8], I32, tag="off32")
                nc.vector.tensor_copy(out=off32[:], in_=off_f[:])

                # ---- gather: 8 indirect DMAs, one 2KB quad row per point;
                # call (c,l) fetches level-l points for queries 32c..32c+32,
                # landing at partition 32*pt + q%32 ----
                G = gp.tile([P, 8, 4 * C], BF16, tag="G", name=f"G_{t}")
                for l in range(2):
                    for c in range(4):
                        s = 4 * l + c
                        nc.gpsimd.indirect_dma_start(
                            out=G[:, s, :], out_offset=None,
                            in_=valq[:, :],
                            in_offset=bass.IndirectOffsetOnAxis(
                                ap=off32[:, s:s + 1], axis=0),
                        )

                # ---- softmax numerator & 1/denominator over 8 attn logits ----
                aw_e = wp.tile([P, 8], F32, tag="aw_e")
                nc.scalar.activation(aw_e[:], lin[:, 16:24],
                                     mybir.ActivationFunctionType.Exp)
                ssum = wp.tile([P, 1], F32, tag="ssum")
                nc.vector.reduce_sum(out=ssum[:], in_=aw_e[:],
                                     axis=mybir.AxisListType.X)
                rinv = wp.tile([P, 1], F32, tag="rinv")
                nc.vector.reciprocal(out=rinv[:], in_=ssum[:])

                # ---- hat weights ----
                # w0 = relu(1-|d|)   = max(min(1-d, 1+d), 0)
                # w1 = relu(1-|d-1|) = max(min(2-d, d), 0)
                d0 = wp.tile([P, 16], F32, tag="d0")
                nc.vector.tensor_sub(out=d0[:], in0=i_all[:], in1=x0f[:])
                f0 = wp.tile([P, 16], F32, tag="f0")
                nc.scalar.activation(f0[:], d0[:],
                                     mybir.ActivationFunctionType.Copy,
                                     bias=0.0, scale=-1.0)
                nc.vector.tensor_scalar_add(out=f0[:], in0=f0[:], scalar1=1.0)
                w0 = wp.tile([P, 16], F32, tag="w0")
                nc.vector.scalar_tensor_tensor(
                    out=w0[:], in0=d0[:], scalar=1.0, in1=f0[:],
                    op0=mybir.AluOpType.add, op1=mybir.AluOpType.min)
                nc.vector.tensor_scalar_max(out=w0[:], in0=w0[:], scalar1=0.0)
                e1 = wp.tile([P, 16], F32, tag="e1")
                nc.vector.tensor_scalar(
                    out=e1[:], in0=d0[:], scalar1=-1.0, scalar2=2.0,
                    op0=mybir.AluOpType.mult, op1=mybir.AluOpType.add)  # 2-d
                w1 = wp.tile([P, 16], F32, tag="w1")
                nc.vector.scalar_tensor_tensor(
                    out=w1[:], in0=d0[:], scalar=0.0, in1=e1[:],
                    op0=mybir.AluOpType.bypass, op1=mybir.AluOpType.min)
                nc.vector.tensor_scalar_max(out=w1[:], in0=w1[:], scalar1=0.0)

                # ---- combine weights: U[q, k, lp] = aw*rinv*wy(k)*wx(k) ----
                U = wp.tile([P, 4, 8], F32, tag="U")
                t0 = wp.tile([P, 8], F32, tag="t0")
                nc.vector.scalar_tensor_tensor(
                    out=t0[:], in0=aw_e[:], scalar=rinv[:, 0:1],
                    in1=w0[:, 1:16:2],
                    op0=mybir.AluOpType.mult, op1=mybir.AluOpType.mult)
                t1 = wp.tile([P, 8], F32, tag="t1")
                nc.vector.scalar_tensor_tensor(
                    out=t1[:], in0=aw_e[:], scalar=rinv[:, 0:1],
                    in1=w1[:, 1:16:2],
                    op0=mybir.AluOpType.mult, op1=mybir.AluOpType.mult)
                nc.vector.tensor_mul(out=U[:, 0, :], in0=t0[:], in1=w0[:, 0:16:2])
                nc.vector.tensor_mul(out=U[:, 1, :], in0=t0[:], in1=w1[:, 0:16:2])
                nc.vector.tensor_mul(out=U[:, 2, :], in0=t1[:], in1=w0[:, 0:16:2])
                nc.vector.tensor_mul(out=U[:, 3, :], in0=t1[:], in1=w1[:, 0:16:2])

                # ---- transpose U into feature-partitioned layout ----
                Ub = wp.tile([P, 32], BF16, tag="Ub")
                nc.scalar.activation(Ub[:], U[:].rearrange("p a b -> p (a b)"),
                                     mybir.ActivationFunctionType.Copy)
                tru = pp1.tile([32, P], BF16, tag="tru")
                nc.tensor.transpose(out=tru[:], in_=Ub[:], identity=idb)
                UT = wp.tile([32, P], BF16, tag="UT")
                nc.vector.tensor_copy(out=UT[:], in_=tru[:])

                # ---- L weights: spread UT rows into block-diag lhsT ----
                Lf_ps = pp1.tile([P, 8, P], F32, tag="Lf")
                for j in range(8):       # j = l*4 + k
                    nc.tensor.matmul(out=Lf_ps[:, j, :],
                                     lhsT=S8[0:32, 128 * j:128 * (j + 1)],
                                     rhs=UT[:], start=True, stop=True)
                Lb = wp.tile([P, 8, P], BF16, tag="Lb")
                for j in range(8):
                    nc.vector.tensor_mul(out=Lb[:, j, :], in0=Lf_ps[:, j, :],
                                         in1=maskg)

                # ---- combine: 32 matmuls accumulate into f32 PSUM ----
                acc_ps = pp2.tile([P, C], F32, tag="acc")
                for c in range(4):
                    for l in range(2):
                        for k in range(4):
                            nc.tensor.matmul(
                                out=acc_ps[32 * c:32 * (c + 1), :],
                                lhsT=Lb[:, 4 * l + k, 32 * c:32 * (c + 1)],
                                rhs=G[:, 4 * l + c, 256 * k:256 * (k + 1)],
                                start=(l == 0 and k == 0),
                                stop=(l == 1 and k == 3),
                                tile_position=(0, 32 * c))

                outsb = wp.tile([P, C], F32, tag="outsb")
                nc.scalar.activation(outsb[:], acc_ps[:],
                                     mybir.ActivationFunctionType.Copy)
                nc.sync.dma_start(out[ts(t, P), :], outsb[:])

                if debug and t == 0:
                    nc.sync.dma_start(dbg["d_lin"][:, :], lin[:])
                    nc.sync.dma_start(dbg["d_iall"][:, :], i_all[:])
                    nc.sync.dma_start(dbg["d_x0f"][:, :], x0f[:])
                    nc.sync.dma_start(dbg["d_U"][:, :],
                                      U[:].rearrange("p a b -> p (a b)"))
                    nc.sync.dma_start(dbg["d_offf"][:, :], off_f[:])
                    dxt = wp.tile([16, P], F32, tag="dxt")
                    nc.vector.tensor_copy(out=dxt[:], in_=xT[:])
                    nc.sync.dma_start(dbg["d_xT"][:, :], dxt[:])
                    dut = wp.tile([32, P], F32, tag="dut")
                    nc.vector.tensor_copy(out=dut[:], in_=UT[:])
                    nc.sync.dma_start(dbg["d_UT"][:, :], dut[:])
                    dlb = wp.tile([P, 8 * P], F32, tag="dlb")
                    nc.vector.tensor_copy(out=dlb[:],
                                          in_=Lb[:].rearrange("p a b -> p (a b)"))
                    nc.sync.dma_start(dbg["d_Lb"][:, :], dlb[:])
                    dg = wp.tile([P, 8 * 1024], F32, tag="dg")
                    nc.vector.tensor_copy(out=dg[:],
                                          in_=G[:].rearrange("p a b -> p (a b)"))
                    nc.sync.dma_start(dbg["d_G"][:, :], dg[:])

    _split_multiwait_noctrl(nc)
    return nc


def make_consts():
    import ml_dtypes
    smat = np.array([W, H, W, H], np.float32)
    cmax = np.tile(np.array([W - 2, H - 2], np.float32), 8)
    # maskg[p, col] = 1 if p%32 == col%32
    pidx = np.arange(P)
    maskg = (pidx[:, None] % 32 == pidx[None, :] % 32).astype(np.float32)

    consb = np.zeros((P, NCB), np.float32)
    # identity
    consb[:, 0:128] = np.eye(P, dtype=np.float32)
    # S8 plane j=4*l+k: S[k*8 + l*4 + pt, 32*pt+g] = 1
    for l in range(2):
        for k in range(4):
            j = 4 * l + k
            Sj = np.zeros((P, 128), np.float32)
            for pt in range(4):
                Sj[k * 8 + l * 4 + pt, 32 * pt:32 * (pt + 1)] = 1.0
            consb[:, 128 + 128 * j:128 + 128 * (j + 1)] = Sj
    # M_l[l*8+pt*2+xy, 32*pt+g] = (1 if xy==0 else W)
    for l in range(2):
        Mf = np.zeros((P, 128), np.float32)
        for pt in range(4):
            Mf[l * 8 + pt * 2 + 0, 32 * pt:32 * (pt + 1)] = 1.0
            Mf[l * 8 + pt * 2 + 1, 32 * pt:32 * (pt + 1)] = float(W)
        consb[:, 1152 + 128 * l:1152 + 128 * (l + 1)] = Mf
    # nvall / onesb
    consb[0, 1408:1536] = float(NV)
    consb[0, 1536:1664] = 1.0
    return smat, cmax, maskg, consb.astype(ml_dtypes.bfloat16)


def pack_consts(b_off, b_attn):
    smat, cmax, maskg, consb = make_consts()
    b_all = np.concatenate([b_off, b_attn]).astype(np.float32)
    consf = np.zeros((P, NCF), np.float32)
    consf[:, 0:4] = smat[None, :]
    consf[:, 4:28] = b_all[None, :]
    consf[:, 28:44] = cmax[None, :]
    consf[:, 44:172] = maskg
    return consf, consb


def make_valq(value):
    """valq[l*NV + y*W + x] = [v(y,x), v(y,x+1), v(y+1,x), v(y+1,x+1)]."""
    import ml_dtypes
    v = np.asarray(value, np.float32).astype(ml_dtypes.bfloat16)
    v = v.reshape(2, H, W, C)
    quad = np.zeros((2, H, W, 4, C), ml_dtypes.bfloat16)
    quad[:, :, :, 0] = v
    quad[:, :, :W - 1, 1] = v[:, :, 1:]
    quad[:, :H - 1, :, 2] = v[:, 1:, :]
    quad[:, :H - 1, :W - 1, 3] = v[:, 1:, 1:]
    return np.ascontiguousarray(quad.reshape(2 * NV, 4 * C))


_CACHED = {}


def _get_nc():
    if 'nc' not in _CACHED:
        _CACHED['nc'] = build_nc()
    return _CACHED['nc']


def kernel(query, key, value, reference_points, spatial_shapes,
           W_off, b_off, W_attn, b_attn):
    import ml_dtypes
    from concourse import bass_utils

    query = np.asarray(query, np.float32)
    reference_points = np.asarray(reference_points, np.float32)
    W_off = np.asarray(W_off, np.float32)
    W_attn = np.asarray(W_attn, np.float32)
    b_off = np.asarray(b_off, np.float32)
    b_attn = np.asarray(b_attn, np.float32)

    nc = _get_nc()

    wall = np.ascontiguousarray(
        np.concatenate([W_off, W_attn], axis=1)).astype(ml_dtypes.bfloat16)
    consf, consb = pack_consts(b_off, b_attn)
    valq = make_valq(value)

    q = query[0].astype(ml_dtypes.bfloat16)     # [Q, C]
    rp = reference_points[0].reshape(QTOT, 4)

    in_maps = []
    for cidx in range(N_CORES):
        sl = slice(cidx * QPC, (cidx + 1) * QPC)
        qc = q[sl]
        rc = rp[sl]
        pad = QPAD - QPC
        qc = np.concatenate([qc, np.broadcast_to(qc[-1:], (pad, C))], 0)
        rc = np.concatenate([rc, np.broadcast_to(rc[-1:], (pad, 4))], 0)
        in_maps.append({
            "qryT": np.ascontiguousarray(qc.T),
            "refp": np.ascontiguousarray(rc),
            "valq": valq,
            "wall": wall,
            "consf": consf,
            "consb": consb,
        })

    _CACHED['in_maps'] = in_maps
    res = bass_utils.run_bass_kernel_spmd(nc, in_maps,
                                          core_ids=list(range(N_CORES)))
    outs = [r["out"][:QPC] for r in res.results]
    full = np.concatenate(outs, axis=0)[None]     # [1, 40000, 256]
    return full.astype(np.float32)


# revision 16
# speedup vs baseline: 1.0158x; 1.0158x over previous
"""Deformable attention Bass kernel for Trainium2, sharded over 8 NeuronCores.

Problem: nn_DeformableAttention (Q=40000 queries, C=256, 2 levels of 128x256
feature maps, 1 head, 4 points/level, bilinear grid-sample w/ zero padding).

Strategy (v2): queries sharded across 8 cores (5000 each, padded to 5120 = 40
tiles of 128); value maps + linears replicated. The key bottleneck of v1 was
SWDGE descriptor generation on the Pool engine: 32 indirect DMAs per tile at
~1.1us fixed cost each. v2 cuts this to 8 per tile by gathering 2KB "quad"
rows from a host-built bf16 buffer valq[l*NV + y*W + x] =
[v(y,x), v(y,x+1), v(y+1,x), v(y+1,x+1)]  (all 4 bilinear corners of one
point in one row). The gather lands corner-partitioned (partition = 16*j2+g,
j2 = (level,point), g = query%16), so the weighted combine runs on the PE as
32 small matmuls accumulating in f32 PSUM; the per-query bilinear weights are
spread into the block-diagonal lhsT layout with two PE transposes plus
constant spread/mask matrices.

Per 128-query tile:
  - PE: offset/attention linears (bf16), two transposes (x0y coords, U
    weights), offset-row matmuls, 4 weight-spread matmuls, 32 combine matmuls
  - DVE/ACT: coords, clamped corner indices, hat weights, softmax, mask-mults
  - Pool: 8 indirect DMA gathers (one per (level,point)), 2KB rows
"""
import sys
import os

sys.path.insert(0, '/opt/trn_rl_repo')

import numpy as np

import concourse.bass as bass
import concourse.mybir as mybir
from concourse.bass import ts
from concourse.tile import TileContext

F32 = mybir.dt.float32
BF16 = mybir.dt.bfloat16
I32 = mybir.dt.int32

N_CORES = 8
H, W = 128, 256
C = 256
NV = H * W          # rows per level feature map
QTOT = 40000
QPC = QTOT // N_CORES       # 5000 queries per core
P = 128                     # partition/tile size
NT_FULL = (QPC + P - 1) // P  # 40 tiles (last padded)
QPAD = NT_FULL * P          # 5120

# consf (f32) column layout: smat(4) | b_all(24) | cmax(16) | maskg(128)
NCF = 4 + 24 + 16 + 128
# consb (bf16) column layout: identity(128) | S8(8*128) | M0(128) | M1(128) |
#                             nvall(128) | onesb(128)
NCB = 128 + 8 * 128 + 128 + 128 + 128 + 128

_WAIT_OP_FROM_MODE = {
    "sem-ge-imm": "sem-ge",
    "sem-eq-imm": "sem-eq",
    "sem-ge": "sem-ge",
    "sem-eq": "sem-eq",
}


def _split_multiwait_noctrl(nc, max_waits=1):
    """This walrus build rejects >1 sync-wait per instruction ("Too many sync
    wait commands"). Hoist extra waits onto standalone single-wait
    EventSemaphore instructions placed immediately before, on the same engine
    (program order on the engine queue preserves semantics)."""
    import bass_rust

    for f in nc.m.functions:
        for b in f.blocks:
            il = list(b.instructions)
            need = [i for i in il
                    if i.sync_info is not None
                    and len(i.sync_info.on_wait) > max_waits]
            if not need:
                continue
            carriers = {}
            created = []
            for inst in need:
                waits = list(inst.sync_info.on_wait)
                cs = []
                for wt in waits[max_waits:]:
                    h = bass_rust.SemaphoreHandle(wt.ant_name, wt.id)
                    ev = nc.engines[inst.engine].wait_op(
                        h, wt.wait_value, _WAIT_OP_FROM_MODE[wt.wait_mode])
                    cs.append(ev.ins)
                    created.append(ev.ins)
                si = inst.sync_info
                si.on_wait = waits[:max_waits]
                inst.sync_info = si
                carriers[inst.name] = cs
            # the new instructions were appended to nc.cur_bb; remove them
            # from wherever they landed, then splice before their drains.
            created_names = {i.name for i in created}
            for f2 in nc.m.functions:
                for b2 in f2.blocks:
                    lst = list(b2.instructions)
                    kept = [i for i in lst if i.name not in created_names]
                    if len(kept) != len(lst):
                        b2.instructions = kept
            out = []
            for inst in list(b.instructions):
                out.extend(carriers.get(inst.name, []))
                out.append(inst)
            b.instructions = out


# cast_mode: 'rne' = hw f32->int32 cast rounds to nearest; 'trunc' = truncates
CAST_MODE = os.environ.get('DEFATT_CAST_MODE', 'rne')


def build_nc(n_tiles=NT_FULL, gather_bufs=6, work_bufs=5, debug=False):
    qpad = n_tiles * P
    nc = bass.Bass("TRN2")
    dbg = {}
    if debug:
        for nm, shp, dt_ in [("d_lin", [P, 24], F32), ("d_iall", [P, 16], F32),
                             ("d_x0f", [P, 16], F32), ("d_U", [P, 32], F32),
                             ("d_offf", [P, 8], F32), ("d_xT", [16, P], F32),
                             ("d_UT", [32, P], F32), ("d_Lb", [P, 8 * P], F32),
                             ("d_G", [P, 8 * 1024], F32)]:
            dbg[nm] = nc.dram_tensor(nm, shp, dt_, kind="ExternalOutput")

    qryT = nc.dram_tensor("qryT", [C, qpad], BF16, kind="ExternalInput")
    refp = nc.dram_tensor("refp", [qpad, 4], F32, kind="ExternalInput")
    valq = nc.dram_tensor("valq", [2 * NV, 4 * C], BF16, kind="ExternalInput")
    wall = nc.dram_tensor("wall", [C, 24], BF16, kind="ExternalInput")
    consf = nc.dram_tensor("consf", [P, NCF], F32, kind="ExternalInput")
    consb = nc.dram_tensor("consb", [P, NCB], BF16, kind="ExternalInput")
    out = nc.dram_tensor("out", [qpad, C], F32, kind="ExternalOutput")

    with TileContext(nc) as tc:
        with (
            tc.tile_pool(name="const", bufs=1) as cp,
            tc.tile_pool(name="work", bufs=work_bufs) as wp,
            tc.tile_pool(name="gather", bufs=gather_bufs) as gp,
            tc.tile_pool(name="ps1", bufs=1, space="PSUM") as pp1,
            tc.tile_pool(name="ps2", bufs=2, space="PSUM") as pp2,
        ):
            # ---- constants, loaded once ----
            ctf = cp.tile([P, NCF], F32)
            nc.scalar.dma_start(ctf[:], consf[:, :])
            smat = ctf[:, 0:4]            # [W, H, W, H]
            b_all = ctf[:, 4:28]          # bias (b_off || b_attn) bcast
            cmax = ctf[:, 28:44]          # clamp max per (l,p,xy): x->W-2, y->H-2
            maskg = ctf[:, 44:172]        # [128,128] delta(p%32 == col%32)

            ctb = cp.tile([P, NCB], BF16)
            nc.scalar.dma_start(ctb[:], consb[:, :])
            idb = ctb[:, 0:128]                      # identity for PE transpose
            S8 = ctb[:, 128:128 + 8 * 128]           # 8 spread matrices [32,128]
            M0 = ctb[0:16, 1152:1280]                # [16,128] offset rows l=0
            M1 = ctb[0:16, 1280:1408]                # [16,128] offset rows l=1
            nvall = ctb[0:1, 1408:1536]              # [1,128] all NV
            onesb = ctb[0:1, 1536:1664]              # [1,128] ones

            wt = cp.tile([P, 2, 24], BF16)   # W_all split into two K-chunks
            nc.scalar.dma_start(
                wt[:], wall.rearrange("(h p) n -> p h n", p=P)[:, :, :])

            qryT_r = qryT.rearrange("(h p) q -> p h q", p=P)

            for t in range(n_tiles):
                # ---- load query (pre-transposed, bf16) + reference points ----
                qT = wp.tile([P, 2, P], BF16, tag="qT")
                nc.sync.dma_start(qT[:], qryT_r[:, :, ts(t, P)])
                rt = wp.tile([P, 4], F32, tag="rt")
                nc.sync.dma_start(rt[:], refp[ts(t, P), :])

                # ---- linears: lin = q @ [W_off || W_attn] + b ----
                lin_ps = pp1.tile([P, 24], F32, tag="lin")
                nc.tensor.matmul(out=lin_ps[:], lhsT=qT[:, 0, :],
                                 rhs=wt[:, 0, :], start=True, stop=False)
                nc.tensor.matmul(out=lin_ps[:], lhsT=qT[:, 1, :],
                                 rhs=wt[:, 1, :], start=False, stop=True)
                lin = wp.tile([P, 24], F32, tag="lin_sb")
                nc.vector.tensor_add(out=lin[:], in0=lin_ps[:], in1=b_all)

                # ---- sample coords: i = ref*scale + off - 0.5 ----
                refsc = wp.tile([P, 4], F32, tag="refsc")
                nc.vector.tensor_mul(out=refsc[:], in0=rt[:], in1=smat)
                i_all = wp.tile([P, 16], F32, tag="i_all")
                for l in range(2):
                    refsc_b = refsc[:, 2 * l:2 * l + 2] \
                        .unsqueeze(1).broadcast_to([P, 4, 2])
                    nc.vector.scalar_tensor_tensor(
                        out=i_all[:, 8 * l:8 * l + 8]
                            .rearrange("p (k x) -> p k x", k=4),
                        in0=lin[:, 8 * l:8 * l + 8]
                            .rearrange("p (k x) -> p k x", k=4),
                        scalar=-0.5, in1=refsc_b,
                        op0=mybir.AluOpType.add, op1=mybir.AluOpType.add)

                # ---- low corner: x0 = clamp(round(i - 0.5), 0, {W,H}-2) ----
                t2 = wp.tile([P, 16], F32, tag="t2")
                nc.vector.tensor_scalar(
                    out=t2[:], in0=i_all[:], scalar1=-0.5, scalar2=0.0,
                    op0=mybir.AluOpType.add, op1=mybir.AluOpType.max)
                nc.vector.tensor_tensor(out=t2[:], in0=t2[:], in1=cmax,
                                        op=mybir.AluOpType.min)
                if CAST_MODE == 'trunc':
                    nc.vector.tensor_scalar_add(out=t2[:], in0=t2[:],
                                                scalar1=0.5)
                x0i = wp.tile([P, 16], I32, tag="x0i")
                nc.vector.tensor_copy(out=x0i[:], in_=t2[:])
                x0f = wp.tile([P, 16], F32, tag="x0f")
                nc.vector.tensor_copy(out=x0f[:], in_=x0i[:])

                # ---- transpose x0y into feature-partitioned layout ----
                x0b = wp.tile([P, 16], BF16, tag="x0b")
                nc.scalar.activation(x0b[:], x0f[:],
                                     mybir.ActivationFunctionType.Copy)
                trx = pp1.tile([16, P], BF16, tag="trx")
                nc.tensor.transpose(out=trx[:], in_=x0b[:], identity=idb)
                xT = wp.tile([16, P], BF16, tag="xT")
                nc.vector.tensor_copy(out=xT[:], in_=trx[:])

                # ---- offsets: offfull_l[32pt+g, q] = l*NV + y0*W + x0 ----
                off_ps = pp1.tile([P, 2, P], F32, tag="off")
                nc.tensor.matmul(out=off_ps[:, 0, :], lhsT=M0, rhs=xT[:],
                                 start=True, stop=True)
                nc.tensor.matmul(out=off_ps[:, 1, :], lhsT=M1, rhs=xT[:],
                                 start=True, stop=False)
                nc.tensor.matmul(out=off_ps[:, 1, :], lhsT=nvall, rhs=onesb,
                                 start=False, stop=True)
                off_f = wp.tile([P, 8], F32, tag="off_f")
                for l in range(2):
                    offm = wp.tile([P, P], F32, tag="offm")
                    nc.vector.tensor_mul(out=offm[:], in0=off_ps[:, l, :],
                                         in1=maskg)
                    nc.vector.tensor_reduce(
                        out=off_f[:, 4 * l:4 * (l + 1)],
                        in_=offm[:].rearrange("p (c g) -> p c g", c=4),
                        axis=mybir.AxisListType.X, op=mybir.AluOpType.add)
                off32 = wp.tile([P, 8], I32, tag="off32")
                nc.vector.tensor_copy(out=off32[:], in_=off_f[:])

                # ---- gather: 8 indirect DMAs, one 2KB quad row per point;
                # call (c,l) fetches level-l points for queries 32c..32c+32,
                # landing at partition 32*pt + q%32 ----
                G = gp.tile([P, 8, 4 * C], BF16, tag="G", name=f"G_{t}")
                for l in range(2):
                    for c in range(4):
                        s = 4 * l + c
                        nc.gpsimd.indirect_dma_start(
                            out=G[:, s, :], out_offset=None,
                            in_=valq[:, :],
                            in_offset=bass.IndirectOffsetOnAxis(
                                ap=off32[:, s:s + 1], axis=0),
                        )

                # ---- softmax numerator & 1/denominator over 8 attn logits ----
                aw_e = wp.tile([P, 8], F32, tag="aw_e")
                nc.scalar.activation(aw_e[:], lin[:, 16:24],
                                     mybir.ActivationFunctionType.Exp)
                ssum = wp.tile([P, 1], F32, tag="ssum")
                nc.vector.reduce_sum(out=ssum[:], in_=aw_e[:],
                                     axis=mybir.AxisListType.X)
                rinv = wp.tile([P, 1], F32, tag="rinv")
                nc.vector.reciprocal(out=rinv[:], in_=ssum[:])

                # ---- hat weights ----
                # w0 = relu(1-|d|)   = max(min(1-d, 1+d), 0)
                # w1 = relu(1-|d-1|) = max(min(2-d, d), 0)
                d0 = wp.tile([P, 16], F32, tag="d0")
                nc.vector.tensor_sub(out=d0[:], in0=i_all[:], in1=x0f[:])
                f0 = wp.tile([P, 16], F32, tag="f0")
                nc.scalar.activation(f0[:], d0[:],
                                     mybir.ActivationFunctionType.Copy,
                                     bias=0.0, scale=-1.0)
                nc.vector.tensor_scalar_add(out=f0[:], in0=f0[:], scalar1=1.0)
                w0 = wp.tile([P, 16], F32, tag="w0")
                nc.vector.scalar_tensor_tensor(
                    out=w0[:], in0=d0[:], scalar=1.0, in1=f0[:],
                    op0=mybir.AluOpType.add, op1=mybir.AluOpType.min)
                nc.vector.tensor_scalar_max(out=w0[:], in0=w0[:], scalar1=0.0)
                e1 = wp.tile([P, 16], F32, tag="e1")
                nc.vector.tensor_scalar(
                    out=e1[:], in0=d0[:], scalar1=-1.0, scalar2=2.0,
                    op0=mybir.AluOpType.mult, op1=mybir.AluOpType.add)  # 2-d
                w1 = wp.tile([P, 16], F32, tag="w1")
                nc.vector.scalar_tensor_tensor(
                    out=w1[:], in0=d0[:], scalar=0.0, in1=e1[:],
                    op0=mybir.AluOpType.bypass, op1=mybir.AluOpType.min)
                nc.vector.tensor_scalar_max(out=w1[:], in0=w1[:], scalar1=0.0)

                # ---- combine weights: U[q, k, lp] = aw*rinv*wy(k)*wx(k) ----
                U = wp.tile([P, 4, 8], F32, tag="U")
                t0 = wp.tile([P, 8], F32, tag="t0")
                nc.vector.scalar_tensor_tensor(
                    out=t0[:], in0=aw_e[:], scalar=rinv[:, 0:1],
                    in1=w0[:, 1:16:2],
                    op0=mybir.AluOpType.mult, op1=mybir.AluOpType.mult)
                t1 = wp.tile([P, 8], F32, tag="t1")
                nc.vector.scalar_tensor_tensor(
                    out=t1[:], in0=aw_e[:], scalar=rinv[:, 0:1],
                    in1=w1[:, 1:16:2],
                    op0=mybir.AluOpType.mult, op1=mybir.AluOpType.mult)
                nc.vector.tensor_mul(out=U[:, 0, :], in0=t0[:], in1=w0[:, 0:16:2])
                nc.vector.tensor_mul(out=U[:, 1, :], in0=t0[:], in1=w1[:, 0:16:2])
                nc.vector.tensor_mul(out=U[:, 2, :], in0=t1[:], in1=w0[:, 0:16:2])
                nc.vector.tensor_mul(out=U[:, 3, :], in0=t1[:], in1=w1[:, 0:16:2])

                # ---- transpose U into feature-partitioned layout ----
                Ub = wp.tile([P, 32], BF16, tag="Ub")
                nc.scalar.activation(Ub[:], U[:].rearrange("p a b -> p (a b)"),
                                     mybir.ActivationFunctionType.Copy)
                tru = pp1.tile([32, P], BF16, tag="tru")
                nc.tensor.transpose(out=tru[:], in_=Ub[:], identity=idb)
                UT = wp.tile([32, P], BF16, tag="UT")
                nc.vector.tensor_copy(out=UT[:], in_=tru[:])

                # ---- L weights: spread UT rows into block-diag lhsT ----
                Lf_ps = pp1.tile([P, 8, P], F32, tag="Lf")
                for j in range(8):       # j = l*4 + k
                    nc.tensor.matmul(out=Lf_ps[:, j, :],
                                     lhsT=S8[0:32, 128 * j:128 * (j + 1)],
                                     rhs=UT[:], start=True, stop=True)
                Lb = wp.tile([P, 8, P], BF16, tag="Lb")
                for j in range(8):
                    nc.vector.tensor_mul(out=Lb[:, j, :], in0=Lf_ps[:, j, :],
                                         in1=maskg)

                # ---- combine: 32 matmuls accumulate into f32 PSUM ----
                acc_ps = pp2.tile([P, C], F32, tag="acc")
                for c in range(4):
                    for l in range(2):
                        for k in range(4):
                            nc.tensor.matmul(
                                out=acc_ps[32 * c:32 * (c + 1), :],
                                lhsT=Lb[:, 4 * l + k, 32 * c:32 * (c + 1)],
                                rhs=G[:, 4 * l + c, 256 * k:256 * (k + 1)],
                                start=(l == 0 and k == 0),
                                stop=(l == 1 and k == 3),
                                tile_position=(0, 32 * c))

                outsb = wp.tile([P, C], F32, tag="outsb")
                nc.scalar.activation(outsb[:], acc_ps[:],
                                     mybir.ActivationFunctionType.Copy)
                nc.sync.dma_start(out[ts(t, P), :], outsb[:])

                if debug and t == 0:
                    nc.sync.dma_start(dbg["d_lin"][:, :], lin[:])
                    nc.sync.dma_start(dbg["d_iall"][:, :], i_all[:])
                    nc.sync.dma_start(dbg["d_x0f"][:, :], x0f[:])
                    nc.sync.dma_start(dbg["d_U"][:, :],
                                      U[:].rearrange("p a b -> p (a b)"))
                    nc.sync.dma_start(dbg["d_offf"][:, :], off_f[:])
                    dxt = wp.tile([16, P], F32, tag="dxt")
                    nc.vector.tensor_copy(out=dxt[:], in_=xT[:])
                    nc.sync.dma_start(dbg["d_xT"][:, :], dxt[:])
                    dut = wp.tile([32, P], F32, tag="dut")
                    nc.vector.tensor_copy(out=dut[:], in_=UT[:])
                    nc.sync.dma_start(dbg["d_UT"][:, :], dut[:])
                    dlb = wp.tile([P, 8 * P], F32, tag="dlb")
                    nc.vector.tensor_copy(out=dlb[:],
                                          in_=Lb[:].rearrange("p a b -> p (a b)"))
                    nc.sync.dma_start(dbg["d_Lb"][:, :], dlb[:])
                    dg = wp.tile([P, 8 * 1024], F32, tag="dg")
                    nc.vector.tensor_copy(out=dg[:],
                                          in_=G[:].rearrange("p a b -> p (a b)"))
                    nc.sync.dma_start(dbg["d_G"][:, :], dg[:])

    _split_multiwait_noctrl(nc)
    return nc


def make_consts():
    import ml_dtypes
    smat = np.array([W, H, W, H], np.float32)
    cmax = np.tile(np.array([W - 2, H - 2], np.float32), 8)
    # maskg[p, col] = 1 if p%32 == col%32
    pidx = np.arange(P)
    maskg = (pidx[:, None] % 32 == pidx[None, :] % 32).astype(np.float32)

    consb = np.zeros((P, NCB), np.float32)
    # identity
    consb[:, 0:128] = np.eye(P, dtype=np.float32)
    # S8 plane j=4*l+k: S[k*8 + l*4 + pt, 32*pt+g] = 1
    for l in range(2):
        for k in range(4):
            j = 4 * l + k
            Sj = np.zeros((P, 128), np.float32)
            for pt in range(4):
                Sj[k * 8 + l * 4 + pt, 32 * pt:32 * (pt + 1)] = 1.0
            consb[:, 128 + 128 * j:128 + 128 * (j + 1)] = Sj
    # M_l[l*8+pt*2+xy, 32*pt+g] = (1 if xy==0 else W)
    for l in range(2):
        Mf = np.zeros((P, 128), np.float32)
        for pt in range(4):
            Mf[l * 8 + pt * 2 + 0, 32 * pt:32 * (pt + 1)] = 1.0
            Mf[l * 8 + pt * 2 + 1, 32 * pt:32 * (pt + 1)] = float(W)
        consb[:, 1152 + 128 * l:1152 + 128 * (l + 1)] = Mf
    # nvall / onesb
    consb[0, 1408:1536] = float(NV)
    consb[0, 1536:1664] = 1.0
    return smat, cmax, maskg, consb.astype(ml_dtypes.bfloat16)


def pack_consts(b_off, b_attn):
    smat, cmax, maskg, consb = make_consts()
    b_all = np.concatenate([b_off, b_attn]).astype(np.float32)
    consf = np.zeros((P, NCF), np.float32)
    consf[:, 0:4] = smat[None, :]
    consf[:, 4:28] = b_all[None, :]
    consf[:, 28:44] = cmax[None, :]
    consf[:, 44:172] = maskg
    return consf, consb


def make_valq(value):
    """valq[l*NV + y*W + x] = [v(y,x), v(y,x+1), v(y+1,x), v(y+1,x+1)]."""
    import ml_dtypes
    v = np.asarray(value, np.float32).astype(ml_dtypes.bfloat16)
    v = v.reshape(2, H, W, C)
    quad = np.zeros((2, H, W, 4, C), ml_dtypes.bfloat16)
    quad[:, :, :, 0] = v
    quad[:, :, :W - 1, 1] = v[:, :, 1:]
    quad[:, :H - 1, :, 2] = v[:, 1:, :]
    quad[:, :H - 1, :W - 1, 3] = v[:, 1:, 1:]
    return np.ascontiguousarray(quad.reshape(2 * NV, 4 * C))


_CACHED = {}


def _get_nc():
    if 'nc' not in _CACHED:
        _CACHED['nc'] = build_nc()
    return _CACHED['nc']


def kernel(query, key, value, reference_points, spatial_shapes,
           W_off, b_off, W_attn, b_attn):
    import ml_dtypes
    from concourse import bass_utils

    query = np.asarray(query, np.float32)
    reference_points = np.asarray(reference_points, np.float32)
    W_off = np.asarray(W_off, np.float32)
    W_attn = np.asarray(W_attn, np.float32)
    b_off = np.asarray(b_off, np.float32)
    b_attn = np.asarray(b_attn, np.float32)

    nc = _get_nc()

    wall = np.ascontiguousarray(
        np.concatenate([W_off, W_attn], axis=1)).astype(ml_dtypes.bfloat16)
    consf, consb = pack_consts(b_off, b_attn)
    valq = make_valq(value)

    q = query[0].astype(ml_dtypes.bfloat16)     # [Q, C]
    rp = reference_points[0].reshape(QTOT, 4)

    in_maps = []
    for cidx in range(N_CORES):
        sl = slice(cidx * QPC, (cidx + 1) * QPC)
        qc = q[sl]
        rc = rp[sl]
        pad = QPAD - QPC
        qc = np.concatenate([qc, np.broadcast_to(qc[-1:], (pad, C))], 0)
        rc = np.concatenate([rc, np.broadcast_to(rc[-1:], (pad, 4))], 0)
        in_maps.append({
            "qryT": np.ascontiguousarray(qc.T),
            "refp": np.ascontiguousarray(rc),
            "valq": valq,
            "wall": wall,
            "consf": consf,
            "consb": consb,
        })

    _CACHED['in_maps'] = in_maps
    res = bass_utils.run_bass_kernel_spmd(nc, in_maps,
                                          core_ids=list(range(N_CORES)))
    outs = [r["out"][:QPC] for r in res.results]
    full = np.concatenate(outs, axis=0)[None]     # [1, 40000, 256]
    return full.astype(np.float32)
